# revision 1
# baseline (speedup 1.0000x reference)
"""MiniRocket feature extraction kernel for Trainium2 (8 NeuronCores, data parallel).

Contract: kernel(**inputs) takes the FULL inputs (as produced by setup_inputs())
and returns the FULL [64, 1344] float32 output. Internally the batch dim is
sharded 8-ways across the 8 NeuronCores; all other tensors are small replicated
constants that are preprocessed on the host into matmul weights / bias tables.

Math (per batch b, dilation d, kernel k, feature f):
    resp[k, l] = sum_{c,j} mask[d,k,c] * kern[k,j] * x[b, l + (j-4)*dil, c]
    feat[k, f] = sum_l w[k,l] * (resp[k,l] > bias[d,k,f])
    out        = (feat - mean) / std
where w is 1/L everywhere (even parity of d_idx+k) or 1/(L-2p) over the
interior [p, L-p) (odd parity, p = 4*dil).

Device mapping (v3 — PSUM-direct counting, half-width 4-slot pipeline):
  - resp computed as matmuls W[72,84]^T @ patch[72,*] per (b, d), where the
    patch holds 9 dilation-shifted copies of x[b] (channels-major) built by a
    single overlapping-AP DMA STRAIGHT FROM DRAM (no SBUF staging of x, no
    SBUF->SBUF traffic). All 32 patch DMAs are issued upfront across the DMA
    queues. Kernels are permuted odd-parity-first so the interior-window trick
    needs only a fixed partition range.
  - Each (b, d) response is produced as TWO [84, 1024] PSUM tiles rotating
    through 4 pool slots (8 banks). The 4-slot slack decouples the PE's
    PSUM-reuse WAR from the two counting engines' reader chain, lifting both
    ACT and DVE from ~71%% to ~86%% busy vs a 2x[84,2048] layout.
  - PPV counting reads resp DIRECTLY FROM PSUM (no fp16 eviction pass):
      * ACT poisons the edge columns of odd-parity rows in PSUM with -1000
        (left edge in half 0, right edge in half 1),
      * f0,f1 on DVE: tensor_scalar(is_gt, add, accum_out) -> direct count,
      * f2,f3 on ACT: Sign(resp - b) with accum -> count = S/2 + L/2 (edge
        poison contributes -1).
    Per-half partial counts land in cnt[:, 0:8] / cnt[:, 8:16] and are folded
    with one tensor_tensor add per count tile before the affine.
  - final affine (count*A + B) folds the PPV weight, mean and std.

walrus in this toolchain encodes at most ONE sync wait per compute/DMA
instruction; _legalize_sync_waits rewrites Tile's emitted waits to fit: a
transitive-closure (vector-clock) min-cover prunes redundant waits, extra
Matmult waits are hoisted onto the preceding Ldweights, and patch-DMA waits
park on earlier free PE slots. CRITICAL semantics baked into the pruner: an
engine's OWN semaphore tick is completion-level knowledge only and must never
propagate through the engine's instruction stream — accumulator-drain aux
ops (and posted writes) lag the next instruction's dispatch on this silicon.
"""

import os
import sys

for _p in (
    "/root/.axon_site",
    "/root/.axon_site/_ro/trn_rl_repo",
    "/root/.axon_site/_ro/pypackages",
    "/opt/trn_rl_repo",
):
    if os.path.isdir(_p) and _p not in sys.path:
        sys.path.append(_p)

import numpy as np

B, L, C = 64, 2048, 8
DILATIONS = (1, 2, 4, 8)
D = 4
K = 84
F = 4
KERNEL_LEN = 9
NCORES = 8
BPC = B // NCORES  # batches per core
PAD = 32  # max shift = 4 * max(dil)
LP = L + 2 * PAD  # padded length

_PROGRAM_CACHE: dict = {}


def _parity_perm(d_idx: int) -> np.ndarray:
    """Kernel order for dilation d: odd-parity (trimmed-window) kernels first."""
    k = np.arange(K)
    parity = (d_idx + k) % 2
    return np.concatenate([k[parity == 1], k[parity == 0]])


def _host_constants(kernels, channel_masks, bias_matrices, feature_mean, feature_std):
    """Build wT [72, 4*84] fp16, cpk [84, 48] fp32, and the kernel permutations."""
    kernels = np.asarray(kernels, np.float32)
    channel_masks = np.asarray(channel_masks, np.float32)
    bias_matrices = np.asarray(bias_matrices, np.float32)
    feature_mean = np.asarray(feature_mean, np.float32).reshape(D, K, F)
    feature_std = np.asarray(feature_std, np.float32).reshape(D, K, F)

    wT_blocks = []
    cpk = np.zeros((K, 48), np.float32)
    perms = []
    for d_idx, dil in enumerate(DILATIONS):
        perm = _parity_perm(d_idx)
        perms.append(perm)
        # W[k', (c,j)] = kern[k, j] * mask[d, k, c], k = perm[k'] — c-major rows
        # to match the patch DMA's (channel, tap) partition order.
        w = channel_masks[d_idx][perm][:, :, None] * kernels[perm][:, None, :]
        wT_blocks.append(w.reshape(K, C * KERNEL_LEN).T.astype(np.float16))  # [72, 84]

        pad = 4 * dil
        w_sel = np.where(np.arange(K) < 42, 1.0 / (L - 2 * pad), 1.0 / L).astype(
            np.float32
        )  # odd-parity rows first 42
        bias_p = bias_matrices[d_idx][perm]  # [84, 4]
        mean_p = feature_mean[d_idx][perm]  # [84, 4]
        std_p = feature_std[d_idx][perm]  # [84, 4]
        # f0,f1 are counted on DVE (is_gt -> direct count); f2,f3 on ACT
        # (Sign accumulate: count = S/2 + L/2, edge poison contributes -1).
        c0 = 4 * d_idx
        cpk[:, c0 : c0 + 2] = bias_p[:, 0:2]
        cpk[:, c0 + 2 : c0 + 4] = -bias_p[:, 2:4]
        cpk[:, 16 + c0 : 16 + c0 + 2] = w_sel[:, None] / std_p[:, 0:2]
        cpk[:, 16 + c0 + 2 : 16 + c0 + 4] = w_sel[:, None] / (2.0 * std_p[:, 2:4])
        cpk[:, 32 + c0 : 32 + c0 + 2] = -mean_p[:, 0:2] / std_p[:, 0:2]
        cpk[:, 32 + c0 + 2 : 32 + c0 + 4] = (
            w_sel[:, None] * (L / 2.0) - mean_p[:, 2:4]
        ) / std_p[:, 2:4]

    wT = np.concatenate(wT_blocks, axis=1)  # [72, 4*84] fp16
    return wT, cpk, perms


def _build_program():
    """Build the Bass/Tile program (same NEFF for all 8 cores)."""
    from contextlib import ExitStack

    import bass_rust
    import concourse.bass as bass
    import concourse.tile as tile
    from concourse import mybir

    f16 = mybir.dt.float16
    f32 = mybir.dt.float32
    A = mybir.AluOpType
    ACT = mybir.ActivationFunctionType

    nc = bass.Bass()
    xT = nc.declare_dram_parameter("xT", [BPC * C, LP], f16, isOutput=False)
    wT = nc.declare_dram_parameter("wT", [72, D * K], f16, isOutput=False)
    cpk = nc.declare_dram_parameter("cpk", [K, 48], f32, isOutput=False)
    out = nc.declare_dram_parameter("out", [BPC, K, 16], f32, isOutput=True)

    def patch_src(b, dil):
        """DRAM view: 9 dilation-shifted [C, L] windows of batch b, c-major."""
        c = xT.ap().copy()
        c.offset = b * C * LP + PAD - 4 * dil
        c.ap = bass_rust.VecI64Pair([[LP, C], [dil, KERNEL_LEN], [1, L]])
        return c

    with tile.TileContext(nc) as tc, ExitStack() as ctx:
        cpool = ctx.enter_context(tc.tile_pool(name="const", bufs=1))
        patch_pool = ctx.enter_context(tc.tile_pool(name="patch", bufs=BPC * D))
        psum_pool = ctx.enter_context(tc.tile_pool(name="psum", bufs=4, space="PSUM"))
        trash_pool = ctx.enter_context(tc.tile_pool(name="trash", bufs=2))
        tra_pool = ctx.enter_context(tc.tile_pool(name="tra", bufs=1))
        cnt_pool = ctx.enter_context(tc.tile_pool(name="cnt", bufs=2))
        cnta_pool = ctx.enter_context(tc.tile_pool(name="cnta", bufs=BPC))
        osb_pool = ctx.enter_context(tc.tile_pool(name="osb", bufs=1))
        scr_pool = ctx.enter_context(tc.tile_pool(name="scr", bufs=1))

        wsb = cpool.tile([72, D * K], f16)
        nc.sync.dma_start(wsb[:], wT.ap())
        csb = cpool.tile([K, 48], f32)
        nc.sync.dma_start(csb[:], cpk.ap())

        # All 32 patch DMAs issued upfront, straight from DRAM (no deps, no
        # slot reuse -> no sync waits on any patch DMA).
        patches = []
        for b in range(BPC):
            for d_idx, dil in enumerate(DILATIONS):
                patch = patch_pool.tile([72, L], f16, name="patch")
                nc.gpsimd.dma_start(patch[:], patch_src(b, dil))
                patches.append(patch)

        osb = osb_pool.tile([K, BPC * 16], f32)
        scr = scr_pool.tile([1, 40], f32)

        # Touch csb from DVE and ACT once so its DMA-completion tick is in
        # both engines' vector clocks; later ops then carry only one sync wait
        # each (walrus encodes at most one per compute instruction).
        nc.vector.tensor_copy(osb[:, 0:1], csb[:, 0:1])
        nc.scalar.activation(scr[0:1, 0:1], csb[0:1, 0:1], ACT.Copy)

        for b in range(BPC):
            cnt = cnt_pool.tile([K, 16], f32)
            cnt_a = cnta_pool.tile([K, 16], f32)
            for d_idx, dil in enumerate(DILATIONS):
                patch = patches[b * D + d_idx]
                pad = 4 * dil
                for h in range(2):
                    ps = psum_pool.tile([K, 1024], f32)
                    for nt in range(2):
                        nc.tensor.matmul(
                            ps[:, nt * 512 : (nt + 1) * 512],
                            lhsT=wsb[:, d_idx * K : (d_idx + 1) * K],
                            rhs=patch[:, h * 1024 + nt * 512 : h * 1024 + (nt + 1) * 512],
                            start=True,
                            stop=True,
                        )
                    # Poison the edge columns of the odd-parity kernels with
                    # -1000 in PSUM: left edge lives in half 0, right in half 1.
                    if h == 0:
                        nc.scalar.activation(
                            ps[0:42, 0:pad], csb[0:42, 0:pad], ACT.Copy,
                            bias=-1000.0, scale=0.0,
                        )
                    else:
                        nc.scalar.activation(
                            ps[0:42, 1024 - pad : 1024], csb[0:42, 0:pad], ACT.Copy,
                            bias=-1000.0, scale=0.0,
                        )

                    trash = trash_pool.tile([K, 1024], f16)
                    trash_a = tra_pool.tile([K, 1024], f16)
                    # f2,f3 on ACT: Sign(resp - b) accumulated (= 2*count - L).
                    for f in range(2, 4):
                        col = 4 * d_idx + f
                        nc.scalar.activation(
                            trash_a[:],
                            ps[:],
                            ACT.Sign,
                            bias=csb[:, col : col + 1],
                            accum_out=cnt_a[
                                :, 8 * h + 2 * d_idx + f - 2 : 8 * h + 2 * d_idx + f - 1
                            ],
                        )
                    # f0,f1 on DVE: direct count via is_gt + accumulate.
                    for f in range(2):
                        col = 4 * d_idx + f
                        nc.vector.tensor_scalar(
                            trash[:],
                            ps[:],
                            csb[:, col : col + 1],
                            None,
                            A.is_gt,
                            A.add,
                            accum_out=cnt[
                                :, 8 * h + 2 * d_idx + f : 8 * h + 2 * d_idx + f + 1
                            ],
                        )

            # Fold the two halves' partial counts (DVE, tiny).
            nc.vector.tensor_tensor(cnt[:, 0:8], cnt[:, 0:8], cnt[:, 8:16], A.add)
            nc.vector.tensor_tensor(cnt_a[:, 0:8], cnt_a[:, 0:8], cnt_a[:, 8:16], A.add)

            def pairs_ap(base_ap, pitch):
                a = base_ap.copy()
                a.ap = bass_rust.VecI64Pair([[pitch, K], [4, 4], [1, 2]])
                return a

            for cnt_t, off in ((cnt, 0), (cnt_a, 2)):
                obp = pairs_ap(osb[:, b * 16 + off : b * 16 + off + 2], BPC * 16)
                cntp = cnt_t[:].copy()
                cntp.ap = bass_rust.VecI64Pair([[16, K], [2, 4], [1, 2]])
                ap_a = pairs_ap(csb[:, 16 + off : 16 + off + 2], 48)
                ap_b = pairs_ap(csb[:, 32 + off : 32 + off + 2], 48)
                nc.vector.tensor_tensor(obp, cntp, ap_a, A.mult)
                nc.vector.tensor_tensor(obp, obp, ap_b, A.add)

        # Single output store: src [84, BPC*16] (k-major) -> DRAM [BPC, 84, 16].
        dst = out.ap().copy()
        dst.ap = bass_rust.VecI64Pair([[16, K], [K * 16, BPC], [1, 16]])
        nc.sync.dma_start(dst, osb[:])

    _legalize_sync_waits(nc, bass_rust)
    return nc


# Map each engine to its own completion-semaphore prefix; an instruction's
# wait on its own engine's semaphore is redundant when paired with other
# waits (the engine's stream is in-order).
_ENG_SEM_PFX = {
    "PE": "PE_",
    "Activation": "Activation_",
    "DVE": "DVE_",
    "Pool": "Pool_",
    "SP": "SP_",
}


def _legalize_sync_waits(nc, bass_rust):
    """walrus encodes at most ONE sync wait per compute/DMA instruction.
    Rewrites, validated in the CoreSim race detector and on hardware:
     1. Drop same-engine self-waits when an instruction holds other waits;
        every such pairing here is transitively covered (the other wait's
        producer itself waited on the self-sem tick).
     2. Hoist extra Matmult waits onto the immediately-preceding Ldweights
        (same engine, program order): a wait satisfied before the Ldweights is
        satisfied before the matmul.
     3. Prune the kernel-tail SP drain: a wait there is redundant when (a) it
        targets a compute engine (that engine's own drain + the all-engine
        barrier already gate the semaphore reset), or (b) some instruction in
        the body already waits for that semaphore's final tick (the consumer
        observed completion, and consumers are covered by (a)). What survives
        (the output-store queue, which nothing consumes) stays on the drain.
    """
    blocks = list(nc.m.functions[0].blocks)
    end_blk = next(b for b in blocks if b.name.endswith("_end"))

    max_waited: dict = {}
    for blk in blocks:
        if blk is end_blk:
            continue
        for inst in blk.instructions:
            si = inst.sync_info
            for w in si.on_wait if si and si.on_wait else []:
                if w.wait_value > max_waited.get(w.ant_name, -1):
                    max_waited[w.ant_name] = w.wait_value

    # Transitive wait pruning over the body block. Engines complete their
    # instructions in order, so before instruction i runs, everything its
    # same-engine predecessor knew (and produced) has happened; a kept wait
    # on (S >= v) additionally implies everything the producer of S's v-th
    # tick knew at completion. DMA enqueue instructions complete
    # asynchronously, so their own sem update is NOT part of the enqueueing
    # stream's knowledge — a DMA-sem wait only inherits the enqueue-time
    # knowledge of the DMACopy.
    body = [b for b in blocks if b is not end_blk and not b.name == "main"]
    know_after: dict = {}  # stream knowledge (excludes own sem: accum aux lag)
    know_full: dict = {}  # completion knowledge (includes own sem updates)
    producers: dict = {}  # sem -> list of (value, inst_idx, is_dma)
    prev_on_engine: dict = {}
    pe_free_slots: list = []
    insts = [i for b in body for i in b.instructions]

    def covered(know, sem, val):
        return know.get(sem, -1) >= val

    def refs(ins_or_outs):
        return {
            getattr(a, "memref", None)
            for a in ins_or_outs
            if getattr(a, "memref", None)
        }

    reads = [refs(i.ins) for i in insts]
    writes = [refs(i.outs) for i in insts]
    accum_on = [
        len(list(i.outs)) >= 2 and i.opcode in ("Activation", "TensorScalarPtr")
        for i in insts
    ]
    eng_of = [str(i.engine).split(".")[-1] for i in insts]

    def hazard(p, c):
        return bool(
            (writes[p] & (reads[c] | writes[c])) or (reads[p] & writes[c])
        )

    for idx, inst in enumerate(insts):
        eng = str(inst.engine).split(".")[-1]
        si = inst.sync_info
        is_dma = inst.opcode == "DMACopy"
        know = dict(know_after.get(prev_on_engine.get(eng), {}))
        waits = list(si.on_wait) if si and si.on_wait else []
        if waits:
            # Tile chains the two compute engines' READS of the same PSUM
            # tile (a pure RAR scheduling artifact that serializes DVE and
            # ACT). Drop a wait ONLY in that provably-safe case: no memref
            # hazard between any producer tick <= v and this instruction,
            # and the wait's own producer is a compute op reading a PSUM ps
            # tile (PSUM tiles are persistent here and PSUM never aliases
            # SBUF, so memref-level analysis is exact for them). All other
            # waits are kept untouched — SBUF tiles can alias at the
            # allocator level, which memref analysis cannot see.
            kept_w = []
            for w in waits:
                drop = False
                if True:
                    kept_w.append(w)
                    continue
                plist = producers.get(w.ant_name, [])
                any_hazard = False
                prod = None
                for v, pidx, pdma in plist:
                    if v > w.wait_value:
                        break
                    prod = (pidx, pdma)
                    if hazard(pidx, idx) or (
                        accum_on[idx]
                        and accum_on[pidx]
                        and eng_of[pidx] == eng
                    ):
                        any_hazard = True
                if prod is not None and not any_hazard:
                    pidx, pdma = prod
                    if not pdma and any(
                        r and r.startswith("ps_") for r in reads[pidx]
                    ):
                        drop = True
                if not drop:
                    kept_w.append(w)
            waits = kept_w
        if waits:
            # knowledge each wait would contribute
            contrib = []
            for w in waits:
                c = {}
                for v, pidx, pdma in producers.get(w.ant_name, []):
                    if v >= w.wait_value:
                        c = dict(know_full.get(pidx, {}))
                        break
                c[w.ant_name] = max(c.get(w.ant_name, -1), w.wait_value)
                contrib.append(c)
            # find the smallest subset of waits whose merged transitive
            # knowledge (plus same-engine knowledge) covers every wait
            from itertools import combinations

            need = [
                i
                for i, w in enumerate(waits)
                if not covered(know, w.ant_name, w.wait_value)
            ]
            best = None
            for sz in range(0, len(need) + 1):
                for sub in combinations(need, sz):
                    merged = dict(know)
                    for i in sub:
                        for s, v in contrib[i].items():
                            if merged.get(s, -1) < v:
                                merged[s] = v
                    if all(
                        covered(merged, waits[i].ant_name, waits[i].wait_value)
                        for i in need
                    ):
                        best = (sub, merged)
                        break
                if best is not None:
                    break
            assert best is not None
            know = best[1]
            waits = [waits[i] for i in best[0]]
        prev = insts[idx - 1] if idx > 0 else None
        if inst.opcode == "Matmult" and len(waits) > 2:
            # Park DMA-queue waits on an earlier free PE slot: a DMA's
            # completion depends on nothing downstream, so waiting earlier
            # cannot deadlock, and patches are prefetched far ahead so it
            # cannot stall either.
            moved = []
            rest = []
            for w in waits:
                if w.ant_name.startswith("DMA") and pe_free_slots:
                    slot = pe_free_slots.pop(0)
                    ssi = slot.sync_info
                    if ssi is None:
                        ssi = bass_rust.SyncInfo(on_wait=[], on_update=[])
                        slot.sync_info = ssi
                    ssi.on_wait = [w]
                    moved.append(w)
                else:
                    rest.append(w)
            waits = rest
        if inst.opcode == "Matmult" and len(waits) > 1:
            assert prev is not None and prev.opcode == "Ldweights", (
                f"matmul {inst.name} has {len(waits)} waits and no "
                f"preceding Ldweights to hoist onto (prev={prev and prev.opcode})"
            )
            psi = prev.sync_info
            if psi is None:
                psi = bass_rust.SyncInfo(on_wait=[], on_update=[])
                prev.sync_info = psi
            psi.on_wait = list(psi.on_wait) + waits[:-1]
            waits = waits[-1:]
            if prev in pe_free_slots:
                pe_free_slots.remove(prev)
        assert len(waits) <= 1, (
            f"{inst.name} {inst.opcode} still has waits "
            f"{[(w.ant_name, w.wait_value) for w in waits]}"
        )
        if si is not None:
            si.on_wait = waits
        elif waits:
            inst.sync_info = bass_rust.SyncInfo(on_wait=waits, on_update=[])
        # record updates (update_value is an INCREMENT; waits are cumulative
        # thresholds, so track running totals per semaphore). An instruction
        # with an accumulator output drains it via a lagging aux op: its sem
        # tick is completion-level knowledge only and must NOT propagate
        # through the engine stream (the next instruction may start first).
        # Plain (non-accum) compute writes land in pipeline order, so their
        # ticks are stream knowledge. DMA enqueues complete asynchronously.
        has_accum = len(list(inst.outs)) >= 2 and inst.opcode in (
            "Activation",
            "TensorScalarPtr",
        )
        full = dict(know)
        if si and si.on_update:
            for u in si.on_update:
                plist = producers.setdefault(u.ant_name, [])
                total = (plist[-1][0] if plist else 0) + u.update_value
                plist.append((total, idx, is_dma))
                if not is_dma:
                    if full.get(u.ant_name, -1) < total:
                        full[u.ant_name] = total
                    pass  # own ticks are completion-level knowledge only
        know_after[idx] = know
        know_full[idx] = full
        prev_on_engine[eng] = idx
        if inst.opcode in ("Matmult", "Ldweights") and not waits:
            pe_free_slots.append(inst)

    # (3) tail drain
    end_insts = list(end_blk.instructions)
    tail = end_insts[0]
    assert tail.opcode == "Drain", f"unexpected end block head {tail.opcode}"
    si = tail.sync_info
    if si and len(si.on_wait) > 1:
        eng_pfx = ("Activation_", "PE_", "DVE_", "Pool_", "SP_")
        keep = [
            w
            for w in si.on_wait
            if not w.ant_name.startswith(eng_pfx)
            and max_waited.get(w.ant_name, -1) < w.wait_value
        ]
        if len(keep) > 1:
            # spill extras onto zero-wait Pool drains before the sem reset
            spill_slots = []
            for inst in end_insts[1:]:
                if inst.opcode == "ISA":
                    break
                isi = inst.sync_info
                if inst.opcode == "Drain" and (not isi or not isi.on_wait):
                    spill_slots.append(inst)
            assert len(spill_slots) >= len(keep) - 1, (
                f"tail drain needs {len(keep)} wait slots, "
                f"only {1 + len(spill_slots)} available"
            )
            for w, slot in zip(keep[1:], spill_slots):
                ssi = slot.sync_info
                if ssi is None:
                    ssi = bass_rust.SyncInfo(on_wait=[], on_update=[])
                    slot.sync_info = ssi
                ssi.on_wait = [w]
            keep = keep[:1]
        si.on_wait = keep


def _get_program():
    if "nc" not in _PROGRAM_CACHE:
        _PROGRAM_CACHE["nc"] = _build_program()
    return _PROGRAM_CACHE["nc"]


def _prep_x(x):
    """[64, 2048, 8] f32 -> per-core [64, 2112] f16 slices (channels-major, padded)."""
    xt = np.ascontiguousarray(np.asarray(x, np.float32).transpose(0, 2, 1))
    xp = np.zeros((B, C, LP), np.float16)
    xp[:, :, PAD : PAD + L] = xt.astype(np.float16)
    return [
        xp[i * BPC : (i + 1) * BPC].reshape(BPC * C, LP) for i in range(NCORES)
    ]


def kernel(
    x,
    kernels,
    channel_masks,
    bias_matrices,
    feature_mean,
    feature_std,
    _trace=False,
    _sim=False,
):
    wT, cpk, perms = _host_constants(
        kernels, channel_masks, bias_matrices, feature_mean, feature_std
    )
    x_slices = _prep_x(x)
    nc = _get_program()

    in_maps = [
        {"xT": x_slices[i], "wT": wT, "cpk": cpk} for i in range(NCORES)
    ]

    if _sim:
        import concourse.bass_interp as bass_interp

        try:
            nc.detect_race_conditions = False
        except Exception:
            pass
        sim = bass_interp.MultiCoreSim(nc, 1)
        sim.cores[0].assign_tensors(in_maps[0])
        sim.simulate()
        dev_outs = [np.array(sim.cores[0].tensor("out"))]
        full = np.zeros((B, 1344), np.float32)
        _scatter(full[:BPC], dev_outs[0], perms)
        _PROGRAM_CACHE["exec_time_ns"] = None
        return full

    if _trace:
        _install_ntff_hook_shim()

    from concourse.bass_utils import run_bass_kernel_spmd

    res = run_bass_kernel_spmd(
        nc,
        in_maps,
        core_ids=list(range(NCORES)),
        trace=_trace,
        trace_cores=list(range(NCORES)) if _trace else None,
    )
    _PROGRAM_CACHE["exec_time_ns"] = res.exec_time_ns
    _PROGRAM_CACHE["mean_exec_time_ns"] = res.mean_exec_time_ns
    _PROGRAM_CACHE["trace"] = res.instructions_and_trace

    full = np.empty((B, 1344), np.float32)
    for i in range(NCORES):
        _scatter(full[i * BPC : (i + 1) * BPC], res.results[i]["out"], perms)
    return full


def _install_ntff_hook_shim():
    """The image's antenv lacks axon_hooks; provide it so run_bass_kernel_spmd
    trace=True can capture NTFF profiles through the axon tunnel."""
    import sys as _sys
    import types

    try:
        from antenv.axon_hooks import get_axon_ntff_profile_hook  # noqa: F401

        return
    except ImportError:
        pass
    from trn_agent_boot.trn_boot import _ntff_profile_via_ctypes

    hook = _ntff_profile_via_ctypes("/opt/axon/libaxon_pjrt.so")
    mod = types.ModuleType("antenv.axon_hooks")
    mod.get_axon_ntff_profile_hook = lambda: hook
    mod.set_axon_ntff_profile_hook = lambda h: None
    _sys.modules["antenv.axon_hooks"] = mod


def _scatter(dst, dev_out, perms):
    """dev_out [BPC, 84, 16] (k' x (d,f)) -> dst [BPC, 1344] in reference order."""
    dev = np.asarray(dev_out, np.float32).reshape(BPC, K, D, F)
    fidx = np.arange(F)[None, :]
    for d_idx in range(D):
        cols = d_idx * (K * F) + perms[d_idx][:, None] * F + fidx  # [84, 4]
        dst[:, cols] = dev[:, :, d_idx, :]



# revision 19
# speedup vs baseline: 1.0606x; 1.0606x over previous
"""MiniRocket feature extraction kernel for Trainium2 (8 NeuronCores, data parallel).

Contract: kernel(**inputs) takes the FULL inputs (as produced by setup_inputs())
and returns the FULL [64, 1344] float32 output. Internally the batch dim is
sharded 8-ways across the 8 NeuronCores; all other tensors are small replicated
constants that are preprocessed on the host into matmul weights / bias tables.

Math (per batch b, dilation d, kernel k, feature f):
    resp[k, l] = sum_{c,j} mask[d,k,c] * kern[k,j] * x[b, l + (j-4)*dil, c]
    feat[k, f] = w[k] * #{l in W_k : resp[k,l] > bias[d,k,f]}
    out        = (feat - mean) / std
where W_k is the full [0,L) window (even parity of d_idx+k, w=1/L) or the
interior [p, L-p) (odd parity, p = 4*dil, w=1/(L-2p)).

Device mapping (v4 -- 128-row packed PSUM, PSUM-direct counting, edge trick):
  - The per-(b,d) responses are packed 336-rows-per-batch into 24 logical
    PSUM tiles [128, 2048] (3 per batch) via quadrant-legal matmul sub-blocks
    (out partition offsets in {0,32,64,96}; <=32-row blocks anywhere, <=64-row
    blocks at {0,64}).  Counting cost is per-COLUMN, so 128-row tiles cut the
    count-op count from 128 to 96 vs the unpacked [84, *] layout.
  - The interior-window (odd-parity) trick is folded into the matmul: each
    patch carries a 73rd row holding the edge-indicator e_d[l] (1 on the
    2*pad edge columns), and the weight matrix gives that row -1e4 for
    odd-parity kernels.  Edge columns of odd rows come out of the matmul
    already poisoned below every bias -- no separate poison ops.
  - PPV counting reads resp DIRECTLY FROM PSUM, one op per (tile, feature):
      * DVE: tensor_scalar(is_gt, add, accum_out) -> direct count,
      * ACT: Sign(resp - b) with accum -> count = S/2 + L/2,
    with the (tile, feature) -> engine assignment chosen so both engines
    carry equal time (ACT ops are ~10% cheaper than DVE ops).
  - Final affine (count*A + B) folds the PPV weight, mean and std; A/B/bias
    tables are host-built per (tile-row, tile-feature-col) so dead partition
    rows (the 48 quadrant-packing crumbs per batch) are simply zeroed.
  - Patch tiles [73, 2048] rotate through 8 slots; slot s always serves
    dilation s%4, so row 72 (the e_d row) is written once upfront and the
    per-(b,d) patch DMA only rewrites rows 0..71 straight from DRAM.

walrus in this toolchain encodes at most ONE sync wait per compute/DMA
instruction; _legalize_sync_waits rewrites Tile's emitted waits to fit: a
transitive-closure (vector-clock) min-cover prunes redundant waits, extra
Matmult waits are hoisted onto the preceding Ldweights, and DMA waits park on
earlier free PE slots.  CRITICAL semantics baked into the pruner: an engine's
OWN semaphore tick is completion-level knowledge only and must never propagate
through the engine's instruction stream -- accumulator-drain aux ops (and
posted writes) lag the next instruction's dispatch on this silicon.
"""

import os
import sys

for _p in (
    "/root/.axon_site",
    "/root/.axon_site/_ro/trn_rl_repo",
    "/root/.axon_site/_ro/pypackages",
    "/opt/trn_rl_repo",
):
    if os.path.isdir(_p) and _p not in sys.path:
        sys.path.append(_p)

import numpy as np

B, L, C = 64, 2048, 8
DILATIONS = (1, 2, 4, 8)
D = 4
K = 84
F = 4
KERNEL_LEN = 9
NCORES = 8
BPC = B // NCORES  # batches per core
PAD = 32  # max shift = 4 * max(dil)
LP = L + 2 * PAD  # padded length
TPB = 3  # logical [128, 2048] tiles per batch
NT = BPC * TPB  # 24 logical tiles per core
EDGE_W = -30000.0  # edge-poison weight on patch row 72 (odd-parity kernels)

# Per-batch quadrant-legal packing:
# (dilation, k0, k1, k1p, tile_in_batch, poffset).  k1p pads the weight block
# with zero columns so every PSUM partition row is matmul-written (the pad
# rows produce zeros and their count outputs are discarded via zeroed A/B).
# <=32-row blocks may sit at {0,32,64,96}; <=64-row at {0,64}; else 0.
PACK = (
    (0, 0, 64, 64, 0, 0),
    (0, 64, 84, 96, 0, 64),
    (1, 0, 32, 32, 0, 96),
    (1, 32, 84, 96, 1, 0),
    (2, 0, 32, 32, 1, 64),
    (2, 32, 64, 64, 1, 96),
    (2, 64, 84, 96, 2, 0),
    (3, 64, 84, 96, 2, 32),
    (3, 0, 64, 64, 2, 64),
)
KP = 96  # padded kernel-column stride per dilation in wT

_PROGRAM_CACHE: dict = {}


def _row_map():
    """(tile_in_batch, partition) -> (d, k) or None, from PACK."""
    m = [[None] * 128 for _ in range(TPB)]
    for d, k0, k1, k1p, t, off in PACK:
        for i in range(k1 - k0):
            m[t][off + i] = (d, k0 + i)
    return m


def _engine_map():
    """(t3, f) -> 'dve' | 'act' assignment, balanced for engine time.

    Baseline f0,f1 -> DVE, f2,f3 -> ACT (48/48).  ACT ops are cheaper
    (~2.23us vs ~2.44us), so shift f1 of 3 of every 8 batch-tiles to ACT
    (-> 45 DVE / 51 ACT).
    """
    eng = {}
    for t in range(NT):
        eng[(t, 0)] = "dve"
        eng[(t, 1)] = "act" if t % 8 == 2 else "dve"
        eng[(t, 2)] = "act"
        eng[(t, 3)] = "act"
    return eng


def _col_orders():
    """Per-engine compact column order: list of (t, f) per engine."""
    eng = _engine_map()
    dve_cols = [(t, f) for t in range(NT) for f in range(F) if eng[(t, f)] == "dve"]
    act_cols = [(t, f) for t in range(NT) for f in range(F) if eng[(t, f)] == "act"]
    return dve_cols, act_cols


def _host_constants(kernels, channel_masks, bias_matrices, feature_mean, feature_std):
    """Build wT [73, D*K] f16 and cst [128, NT*4 * 4] f32 (bias, A_dve, A_act, B)."""
    kernels = np.asarray(kernels, np.float32)
    channel_masks = np.asarray(channel_masks, np.float32)
    bias_matrices = np.asarray(bias_matrices, np.float32)
    feature_mean = np.asarray(feature_mean, np.float32).reshape(D, K, F)
    feature_std = np.asarray(feature_std, np.float32).reshape(D, K, F)

    # weights: W[(c,j), k] = mask[d,k,c]*kern[k,j], c-major rows; row 72 =
    # EDGE_W for odd-parity kernels (their edge columns must count as "below
    # every bias" / Sign=-1).  Columns K..KP per dilation are zero pads.
    wT = np.zeros((73, D * KP), np.float16)
    for d_idx in range(D):
        w = channel_masks[d_idx][:, :, None] * kernels[:, None, :]  # [K, C, 9]
        wT[0:72, d_idx * KP : d_idx * KP + K] = (
            w.reshape(K, C * KERNEL_LEN).T.astype(np.float16)
        )
        parity_odd = (d_idx + np.arange(K)) % 2 == 1
        wT[72, d_idx * KP : d_idx * KP + K] = np.where(
            parity_odd, EDGE_W, 0.0
        ).astype(np.float16)

    rows = _row_map()
    dve_cols, act_cols = _col_orders()
    nd, na = len(dve_cols), len(act_cols)
    bias_d = np.zeros((128, nd), np.float32)
    bias_a = np.zeros((128, na), np.float32)
    a_d = np.zeros((128, nd), np.float32)
    a_a = np.zeros((128, na), np.float32)
    b_d = np.zeros((128, nd), np.float32)
    b_a = np.zeros((128, na), np.float32)
    for cols, is_dve in ((dve_cols, True), (act_cols, False)):
        for i, (t, f) in enumerate(cols):
            t3 = t % TPB
            for p in range(128):
                dk = rows[t3][p]
                if dk is None:
                    continue
                d_idx, k = dk
                pad = 4 * DILATIONS[d_idx]
                odd = (d_idx + k) % 2 == 1
                w_sel = 1.0 / (L - 2 * pad) if odd else 1.0 / L
                bb = bias_matrices[d_idx, k, f]
                mm = feature_mean[d_idx, k, f]
                ss = feature_std[d_idx, k, f]
                if is_dve:
                    # cnt = #{resp > b}; feat = w*cnt; out = (feat-m)/s
                    bias_d[p, i] = bb
                    a_d[p, i] = w_sel / ss
                    b_d[p, i] = -mm / ss
                else:
                    # S = sum Sign(resp - b); cnt = S/2 + L/2 (edge poison
                    # contributes -1 like a below-bias sample)
                    bias_a[p, i] = -bb  # ACT bias is ADDED: Sign(resp + (-b))
                    a_a[p, i] = w_sel / (2.0 * ss)
                    b_a[p, i] = (w_sel * (L / 2.0) - mm) / ss
    cst = np.concatenate([bias_d, bias_a, a_d, a_a, b_d, b_a], axis=1)

    # edge rows e_d[l] per dilation
    ebl = np.zeros((D, L), np.float16)
    for d_idx, dil in enumerate(DILATIONS):
        pad = 4 * dil
        ebl[d_idx, :pad] = 1.0
        ebl[d_idx, L - pad :] = 1.0
    return wT, cst, ebl


def _build_program():
    """Build the Bass/Tile program (same NEFF for all 8 cores)."""
    from contextlib import ExitStack

    import bass_rust
    import concourse.bass as bass
    import concourse.tile as tile
    from concourse import mybir

    f16 = mybir.dt.float16
    f32 = mybir.dt.float32
    A = mybir.AluOpType
    ACT = mybir.ActivationFunctionType

    dve_cols, act_cols = _col_orders()
    nd, na = len(dve_cols), len(act_cols)
    ncol = nd + na
    op_idx = {}
    for i, tf in enumerate(dve_cols):
        op_idx[tf] = ("dve", i)
    for i, tf in enumerate(act_cols):
        op_idx[tf] = ("act", i)

    nc = bass.Bass()
    xT = nc.declare_dram_parameter("xT", [BPC * C, LP], f16, isOutput=False)
    wT = nc.declare_dram_parameter("wT", [73, D * KP], f16, isOutput=False)
    ebl = nc.declare_dram_parameter("ebl", [D, L], f16, isOutput=False)
    cst = nc.declare_dram_parameter("cst", [128, 3 * ncol], f32, isOutput=False)
    out = nc.declare_dram_parameter("out", [128, ncol], f32, isOutput=True)

    def patch_src(b, dil):
        """DRAM view: 9 dilation-shifted [C, L] windows of batch b, c-major."""
        c = xT.ap().copy()
        c.offset = b * C * LP + PAD - 4 * dil
        c.ap = bass_rust.VecI64Pair([[LP, C], [dil, KERNEL_LEN], [1, L]])
        return c

    NSLOT = 8  # patch slots; slot s always serves dilation s%4

    with tile.TileContext(nc) as tc, ExitStack() as ctx:
        cpool = ctx.enter_context(tc.tile_pool(name="const", bufs=1))
        patch_pool = ctx.enter_context(tc.tile_pool(name="patch", bufs=1))
        psum_pool = ctx.enter_context(tc.tile_pool(name="psum", bufs=2, space="PSUM"))
        tr_pool = ctx.enter_context(tc.tile_pool(name="tr", bufs=4))
        tra_pool = ctx.enter_context(tc.tile_pool(name="tra", bufs=4))
        cnt_pool = ctx.enter_context(tc.tile_pool(name="cnt", bufs=1))
        osb_pool = ctx.enter_context(tc.tile_pool(name="osb", bufs=1))

        wsb = cpool.tile([73, D * KP], f16)
        nc.sync.dma_start(wsb[:], wT.ap())
        csb = cpool.tile([128, 3 * ncol], f32)
        nc.sync.dma_start(csb[:], cst.ap())

        # patch slots; write the e-row of each slot once upfront
        patches = [
            patch_pool.tile([73, L], f16, name=f"patch{s}") for s in range(NSLOT)
        ]
        for s in range(NSLOT):
            esrc = ebl.ap().copy()
            esrc.offset = (s % D) * L
            esrc.ap = bass_rust.VecI64Pair([[L, 1], [1, L]])
            nc.sync.dma_start(patches[s][72:73, :], esrc)

        cnt_d = cnt_pool.tile([128, nd], f32)
        cnt_a = cnt_pool.tile([128, na], f32)
        scr_d = cnt_pool.tile([128, 1], f32)
        scr_a = cnt_pool.tile([128, 1], f32)
        tmp_a = cnt_pool.tile([128, na], f32)
        osb = osb_pool.tile([128, ncol], f32)

        # Touch csb once from DVE and ACT so its DMA-completion tick is in
        # both engines' vector clocks; later ops then carry at most one wait.
        nc.vector.tensor_copy(scr_d[:], csb[:, 0:1])
        nc.scalar.activation(scr_a[:], csb[0:128, 0:1], ACT.Copy)

        # patch DMAs for the first NSLOT (b,d) pairs upfront
        def issue_patch(b, d_idx):
            s = (b * D + d_idx) % NSLOT
            nc.sync.dma_start(patches[s][0:72, :], patch_src(b, DILATIONS[d_idx]))

        for b in range(2):
            for d_idx in range(D):
                issue_patch(b, d_idx)

        for b in range(BPC):
            for t3 in range(TPB):
                t = b * TPB + t3
                ps = psum_pool.tile([128, 2048], f32, name="ps")
                for d_idx, k0, k1, k1p, tt, off in PACK:
                    if tt != t3:
                        continue
                    patch = patches[(b * D + d_idx) % NSLOT]
                    for ch in range(4):
                        nc.tensor.matmul(
                            ps[off : off + (k1p - k0), ch * 512 : (ch + 1) * 512],
                            lhsT=wsb[:, d_idx * KP + k0 : d_idx * KP + k1p],
                            rhs=patch[:, ch * 512 : (ch + 1) * 512],
                            start=True,
                            stop=True,
                            tile_position=(0, off),
                        )
                # prefetch: the patches this tile finished with
                # (tile t3 completes dilations per PACK; prefetch 2 batches out)
                if t3 == TPB - 1 and b + 2 < BPC:
                    for d_idx in range(D):
                        issue_patch(b + 2, d_idx)

                for f in range(F):
                    which, i = op_idx[(t, f)]
                    if which == "dve":
                        trash = tr_pool.tile([128, 2048], f16, name="trash")
                        nc.vector.tensor_scalar(
                            trash[:],
                            ps[:],
                            csb[:, i : i + 1],
                            None,
                            A.is_gt,
                            A.add,
                            accum_out=cnt_d[:, i : i + 1],
                        )
                    else:
                        trash_a = tra_pool.tile([128, 2048], f16, name="trash_a")
                        nc.scalar.activation(
                            trash_a[:],
                            ps[:],
                            ACT.Sign,
                            bias=csb[:, nd + i : nd + i + 1],
                            accum_out=cnt_a[:, i : i + 1],
                        )

        # affine per engine block: osb = cnt*A + B  (cols [0:nd] DVE-owned
        # (t,f), cols [nd:ncol] ACT-owned; host unscrambles)
        nc.vector.tensor_tensor(
            cnt_d[:], cnt_d[:], csb[:, ncol : ncol + nd], A.mult
        )
        nc.vector.tensor_tensor(
            osb[:, 0:nd], cnt_d[:], csb[:, 2 * ncol : 2 * ncol + nd], A.add
        )
        nc.vector.tensor_tensor(
            tmp_a[:], cnt_a[:], csb[:, ncol + nd : 2 * ncol], A.mult
        )
        nc.vector.tensor_tensor(
            osb[:, nd:ncol], tmp_a[:], csb[:, 2 * ncol + nd : 3 * ncol], A.add
        )

        nc.sync.dma_start(out.ap(), osb[:])

    _legalize_sync_waits(nc, bass_rust)
    return nc


def _legalize_sync_waits(nc, bass_rust):
    """walrus encodes at most ONE sync wait per compute/DMA instruction.
    Rewrites, validated in the CoreSim race detector and on hardware:
     1. Transitive-closure (vector-clock) min-cover prunes redundant waits.
     2. Extra Matmult waits hoist onto the immediately-preceding Ldweights.
     3. Remaining multi-waits on DMAs park on earlier free PE slots.
     4. Kernel-tail SP drain waits prune to (at most) the output-store queue.
    """
    blocks = list(nc.m.functions[0].blocks)
    end_blk = next(b for b in blocks if b.name.endswith("_end"))

    max_waited: dict = {}
    for blk in blocks:
        if blk is end_blk:
            continue
        for inst in blk.instructions:
            si = inst.sync_info
            for w in si.on_wait if si and si.on_wait else []:
                if w.wait_value > max_waited.get(w.ant_name, -1):
                    max_waited[w.ant_name] = w.wait_value

    body = [b for b in blocks if b is not end_blk and not b.name == "main"]
    know_after: dict = {}  # stream knowledge (excludes own sem: accum aux lag)
    know_full: dict = {}  # completion knowledge (includes own sem updates)
    producers: dict = {}  # sem -> list of (value, inst_idx, is_dma)
    prev_on_engine: dict = {}
    eng_stream: dict = {}  # engine -> its instructions in program order
    insts = [i for b in body for i in b.instructions]

    def covered(know, sem, val):
        return know.get(sem, -1) >= val

    for idx, inst in enumerate(insts):
        eng = str(inst.engine).split(".")[-1]
        si = inst.sync_info
        is_dma = inst.opcode == "DMACopy"
        know = dict(know_after.get(prev_on_engine.get(eng), {}))
        waits = list(si.on_wait) if si and si.on_wait else []
        if waits:
            # knowledge each wait would contribute
            contrib = []
            for w in waits:
                c = {}
                for v, pidx, pdma in producers.get(w.ant_name, []):
                    if v >= w.wait_value:
                        c = dict(know_full.get(pidx, {}))
                        break
                c[w.ant_name] = max(c.get(w.ant_name, -1), w.wait_value)
                contrib.append(c)
            # smallest subset of waits whose merged transitive knowledge
            # (plus same-engine knowledge) covers every wait
            from itertools import combinations

            need = [
                i
                for i, w in enumerate(waits)
                if not covered(know, w.ant_name, w.wait_value)
            ]
            best = None
            for sz in range(0, len(need) + 1):
                for sub in combinations(need, sz):
                    merged = dict(know)
                    for i in sub:
                        for s, v in contrib[i].items():
                            if merged.get(s, -1) < v:
                                merged[s] = v
                    if all(
                        covered(merged, waits[i].ant_name, waits[i].wait_value)
                        for i in need
                    ):
                        best = (sub, merged)
                        break
                if best is not None:
                    break
            assert best is not None
            know = best[1]
            waits = [waits[i] for i in best[0]]
        if len(waits) > 1:
            # Hoist extra waits onto recent wait-free instructions of the
            # SAME engine (engines execute in order, so a wait satisfied
            # before an earlier instruction is satisfied before this one).
            # The walk-back is bounded to the last 8 same-engine
            # instructions: the producers of any wait carried here depend
            # only on work that precedes that window (counts of tile t-2,
            # patch DMAs prefetched 2 batches = 6 tiles ahead), so parking
            # a wait there cannot create a cycle.
            eng_insts = eng_stream.get(eng, [])
            for j in range(len(eng_insts) - 1, max(-1, len(eng_insts) - 9), -1):
                if len(waits) <= 1:
                    break
                cand = eng_insts[j]
                csi = cand.sync_info
                if csi is not None and csi.on_wait:
                    continue
                w = waits.pop(0)
                if csi is None:
                    csi = bass_rust.SyncInfo(on_wait=[], on_update=[])
                    cand.sync_info = csi
                csi.on_wait = [w]
        assert len(waits) <= 1, (
            f"{inst.name} {inst.opcode} still has waits "
            f"{[(w.ant_name, w.wait_value) for w in waits]}"
        )
        if si is not None:
            si.on_wait = waits
        elif waits:
            inst.sync_info = bass_rust.SyncInfo(on_wait=waits, on_update=[])
        # record updates (update_value is an INCREMENT; waits are cumulative
        # thresholds, so track running totals per semaphore). An instruction
        # with an accumulator output drains it via a lagging aux op: its sem
        # tick is completion-level knowledge only and must NOT propagate
        # through the engine stream (the next instruction may start first).
        # DMA enqueues complete asynchronously.
        full = dict(know)
        if si and si.on_update:
            for u in si.on_update:
                plist = producers.setdefault(u.ant_name, [])
                total = (plist[-1][0] if plist else 0) + u.update_value
                plist.append((total, idx, is_dma))
                if not is_dma:
                    if full.get(u.ant_name, -1) < total:
                        full[u.ant_name] = total
        know_after[idx] = know
        know_full[idx] = full
        prev_on_engine[eng] = idx
        eng_stream.setdefault(eng, []).append(inst)

    # (4) tail drain
    end_insts = list(end_blk.instructions)
    tail = end_insts[0]
    assert tail.opcode == "Drain", f"unexpected end block head {tail.opcode}"
    si = tail.sync_info
    if si and len(si.on_wait) > 1:
        eng_pfx = ("Activation_", "PE_", "DVE_", "Pool_", "SP_")
        keep = [
            w
            for w in si.on_wait
            if not w.ant_name.startswith(eng_pfx)
            and max_waited.get(w.ant_name, -1) < w.wait_value
        ]
        if len(keep) > 1:
            # spill extras onto zero-wait drains before the sem reset
            spill_slots = []
            for inst in end_insts[1:]:
                if inst.opcode == "ISA":
                    break
                isi = inst.sync_info
                if inst.opcode == "Drain" and (not isi or not isi.on_wait):
                    spill_slots.append(inst)
            assert len(spill_slots) >= len(keep) - 1, (
                f"tail drain needs {len(keep)} wait slots, "
                f"only {1 + len(spill_slots)} available"
            )
            for w, slot in zip(keep[1:], spill_slots):
                ssi = slot.sync_info
                if ssi is None:
                    ssi = bass_rust.SyncInfo(on_wait=[], on_update=[])
                    slot.sync_info = ssi
                ssi.on_wait = [w]
            keep = keep[:1]
        si.on_wait = keep


def _get_program():
    if "nc" not in _PROGRAM_CACHE:
        _PROGRAM_CACHE["nc"] = _build_program()
    return _PROGRAM_CACHE["nc"]


def _prep_x(x):
    """[64, 2048, 8] f32 -> per-core [64, 2112] f16 slices (channels-major, padded)."""
    xt = np.ascontiguousarray(np.asarray(x, np.float32).transpose(0, 2, 1))
    xp = np.zeros((B, C, LP), np.float16)
    xp[:, :, PAD : PAD + L] = xt.astype(np.float16)
    return [
        xp[i * BPC : (i + 1) * BPC].reshape(BPC * C, LP) for i in range(NCORES)
    ]


def kernel(
    x,
    kernels,
    channel_masks,
    bias_matrices,
    feature_mean,
    feature_std,
    _trace=False,
    _sim=False,
):
    wT, cst, ebl = _host_constants(
        kernels, channel_masks, bias_matrices, feature_mean, feature_std
    )
    x_slices = _prep_x(x)
    nc = _get_program()

    in_maps = [
        {"xT": x_slices[i], "wT": wT, "ebl": ebl, "cst": cst}
        for i in range(NCORES)
    ]

    if _sim:
        import concourse.bass_interp as bass_interp

        try:
            nc.detect_race_conditions = False
        except Exception:
            pass
        sim = bass_interp.MultiCoreSim(nc, 1)
        sim.cores[0].assign_tensors(in_maps[0])
        sim.simulate()
        dev_outs = [np.array(sim.cores[0].tensor("out"))]
        full = np.zeros((B, 1344), np.float32)
        _scatter(full[:BPC], dev_outs[0])
        _PROGRAM_CACHE["exec_time_ns"] = None
        return full

    if _trace:
        _install_ntff_hook_shim()

    from concourse.bass_utils import run_bass_kernel_spmd

    res = run_bass_kernel_spmd(
        nc,
        in_maps,
        core_ids=list(range(NCORES)),
        trace=_trace,
        trace_cores=list(range(NCORES)) if _trace else None,
    )
    _PROGRAM_CACHE["exec_time_ns"] = res.exec_time_ns
    _PROGRAM_CACHE["mean_exec_time_ns"] = res.mean_exec_time_ns
    _PROGRAM_CACHE["trace"] = res.instructions_and_trace

    full = np.empty((B, 1344), np.float32)
    for i in range(NCORES):
        _scatter(full[i * BPC : (i + 1) * BPC], res.results[i]["out"])
    return full


def _install_ntff_hook_shim():
    """The image's antenv lacks axon_hooks; provide it so run_bass_kernel_spmd
    trace=True can capture NTFF profiles through the axon tunnel."""
    import sys as _sys
    import types

    try:
        from antenv.axon_hooks import get_axon_ntff_profile_hook  # noqa: F401

        return
    except ImportError:
        pass
    from trn_agent_boot.trn_boot import _ntff_profile_via_ctypes

    hook = _ntff_profile_via_ctypes("/opt/axon/libaxon_pjrt.so")
    mod = types.ModuleType("antenv.axon_hooks")
    mod.get_axon_ntff_profile_hook = lambda: hook
    mod.set_axon_ntff_profile_hook = lambda h: None
    _sys.modules["antenv.axon_hooks"] = mod


def _scatter_index():
    """Precompute (col, partition) -> flat output index maps per device col."""
    if "scatter" in _PROGRAM_CACHE:
        return _PROGRAM_CACHE["scatter"]
    rows = _row_map()
    dve_cols, act_cols = _col_orders()
    all_cols = list(dve_cols) + list(act_cols)
    # for each device column c and partition p: (b, feature_flat) or -1
    ncol = len(all_cols)
    b_of = np.full((ncol,), -1, np.int64)
    feat_of = np.full((ncol, 128), -1, np.int64)
    for c, (t, f) in enumerate(all_cols):
        b_of[c] = t // TPB
        t3 = t % TPB
        for p in range(128):
            dk = rows[t3][p]
            if dk is None:
                continue
            d_idx, k = dk
            feat_of[c, p] = d_idx * K * F + k * F + f
    _PROGRAM_CACHE["scatter"] = (b_of, feat_of)
    return b_of, feat_of


def _scatter(dst, dev_out):
    """dev_out [128, nd+na] -> dst [BPC, 1344] in reference order."""
    dev = np.asarray(dev_out, np.float32)
    b_of, feat_of = _scatter_index()
    for c in range(feat_of.shape[0]):
        m = feat_of[c] >= 0
        dst[b_of[c], feat_of[c][m]] = dev[m, c]
